# revision 7
# baseline (speedup 1.0000x reference)
"""Trainium2 Bass kernel for nn_Attention_spd (dense transformer attention
with spd-modulated bias), sharded batch-parallel across 8 NeuronCores.

Per batch b (one core each):
    qkv = x @ W_qkv ; q,k,v heads of 64
    dots = q @ k^T * DH**-0.5                       [h, n, m]
    bias = normalize(dots*spd, axis=-1) * ||dots||  (row-wise L2)
    attn = softmax(dots + bias) * head_mask_scale
    out  = (attn @ v) merged @ W_out + b_out

Identities used on device:
    bias = (dots*spd) * rho[n],  rho = ||dots|| / max(||dots*spd||, 1e-12)
    z    = dots + P*rho,         P = dots*spd
    softmax denominator via ACT-exp accumulate; 1/den applied to E.
    ||dots||^2 via the Gram trick: u[n] = q_n^T (k^T k) q_n.
    rho = exp(0.5*(ln u - ln s))  (keeps ACT on one table set).

Precision plan: q/k projection fp32; QK^T / t / output proj in fp32r
(tf32-like, 4x faster than fp32 on the PE); v + Gram path bf16-ish;
exp/attn/EV in bf16. Host folds DH**-0.5 into the k columns of W_qkv and
head_mask * H/sum(mask) into the rows of W_out; x passed pre-transposed.
"""

import numpy as np
from contextlib import ExitStack
DBG = {}  # auxiliary outputs; the mid-kernel DMA reads also act as a phase barrier (scheduling workaround)

import concourse.bass as bass
import concourse.tile as tile
import concourse.mybir as mybir
from concourse.bass_utils import run_bass_kernel_spmd
from concourse.vector_clock import ScopedClock

# ---------------- problem constants (hardcoded) ----------------
B, N, DIM, H, DH = 8, 1024, 512, 8, 64
INNER = H * DH            # 512
SCALE = DH ** -0.5
P = 128                   # SBUF partitions
NT = N // P               # 8 n-tiles (and m-tiles)
KD = DIM // P             # 4 contraction subtiles over DIM
AF = mybir.ActivationFunctionType
ALU = mybir.AluOpType
F32 = mybir.dt.float32
F32R = mybir.dt.float32r
BF16 = mybir.dt.bfloat16

# ---------------- walrus workaround patches ----------------
# The walrus build in this container rejects instructions with more than one
# sync-wait command. Split excess waits onto same-engine NoOps, and spread
# the kernel-tail Drain's waits over extra Drains.
_MAX_WAITS = 1
_SKIP_TYPES = (
    "TileBranchInst",
    "BassTileLoopBlock",
    "BassTileConditionalBlock",
    "BassTileCriticalSection",
)


def _split_waits(nc, ordered):
    for _bb, insts in ordered.items():
        new_list = []
        for inst in insts:
            si = getattr(inst, "sync_info", None)
            if (
                si is not None
                and si.on_wait
                and len(si.on_wait) > _MAX_WAITS
                and type(inst).__name__ not in _SKIP_TYPES
            ):
                waits = list(si.on_wait)
                extra, keep = waits[:-_MAX_WAITS], waits[-_MAX_WAITS:]
                for j in range(0, len(extra), _MAX_WAITS):
                    nop = mybir.InstNoOp(
                        name=nc.get_next_instruction_name(),
                        sync_info=mybir.SyncInfo(
                            on_wait=extra[j : j + _MAX_WAITS], on_update=[]
                        ),
                        bass_nofuse=True,
                        engine=inst.engine,
                    )
                    new_list.append(nop)
                inst.sync_info = mybir.SyncInfo(on_wait=keep, on_update=si.on_update)
            new_list.append(inst)
        insts[:] = new_list


_orig_lower = tile.TileContext._lower_ordered_insts


def _patched_lower(self, ordered):
    _split_waits(self.nc, ordered)
    return _orig_lower(self, ordered)


def _patched_drain_and_barrier(self, tick_clock, wait_clock):
    nc = self.nc
    drain_inst = nc.sync.drain()
    wait_clock.add_sem_waits(
        drain_inst.ins, ScopedClock({None: tick_clock.global_clock})
    )
    waits = list(drain_inst.ins.sync_info.on_wait)
    if len(waits) > 1:
        drain_inst.ins.sync_info = mybir.SyncInfo(on_wait=waits[:1], on_update=[])
        for w in waits[1:]:
            extra = nc.sync.drain()
            extra.ins.sync_info = mybir.SyncInfo(on_wait=[w], on_update=[])
    nc.all_engine_barrier()
    assert self.sems is not None
    popped = nc._tile_sem_poison_stack.pop()
    assert popped is self._sem_poison
    nc.clear_and_free_semaphores(list(self.sems.allocated().values()))
    nc.all_engine_barrier()


def _apply_patches():
    if tile.TileContext._lower_ordered_insts is not _patched_lower:
        tile.TileContext._lower_ordered_insts = _patched_lower
        tile.TileContext._drain_and_barrier = _patched_drain_and_barrier


# ---------------- device kernel ----------------

def _build_bass():
    _apply_patches()
    nc = bass.Bass(
        "TRN2", target_bir_lowering=False, debug=False, enable_asserts=False
    )
    xt = nc.dram_tensor("xt", (DIM, N), F32, kind="ExternalInput").ap()
    xtr = nc.dram_tensor("xtr", (DIM, N), F32R, kind="ExternalInput").ap()
    spd = nc.dram_tensor("spd", (N, N), F32, kind="ExternalInput").ap()
    wqk = nc.dram_tensor("wqk", (DIM, 2 * INNER), F32, kind="ExternalInput").ap()
    wqkr = nc.dram_tensor("wqkr", (DIM, 2 * INNER), F32R, kind="ExternalInput").ap()
    wvr = nc.dram_tensor("wvr", (DIM, INNER), F32R, kind="ExternalInput").ap()
    woutr = nc.dram_tensor("woutr", (INNER, DIM), F32R, kind="ExternalInput").ap()
    boutr = nc.dram_tensor("boutr", (1, DIM), F32R, kind="ExternalInput").ap()
    onesr = nc.dram_tensor("onesr", (1, P), F32R, kind="ExternalInput").ap()
    y = nc.dram_tensor("y", (N, DIM), F32, kind="ExternalOutput").ap()
    for nm, shp in [("o_qkT", (P, 8, N)), ("o_u", (P, H * NT)),
                    ("o_g", (P, H // 2, DH)), ("o_ssq0", (P, NT)), ("o_rho0", (P, NT)),
                    ("o_z00", (P, N)), ("o_den0", (P, NT)),
                    ("o_merged", (P, KD, N))]:
        DBG[nm] = nc.dram_tensor(nm, shp, F32, kind="ExternalOutput").ap()
    for nm, shp in [("o_v", (P, NT, INNER)), ("o_e00", (P, N)), ("o_et0", (P, NT, N))]:
        DBG[nm] = nc.dram_tensor(nm, shp, BF16, kind="ExternalOutput").ap()

    with tile.TileContext(nc) as tc, ExitStack() as ctx:
        _emit(nc, tc, ctx, xt, xtr, spd, wqk, wqkr, wvr, woutr, boutr, onesr, y)
    return nc


def _emit(nc, tc, ctx, xt, xtr, spd, wqk, wqkr, wvr, woutr, boutr, onesr, y):
    # ------- persistent pools -------
    const_p = ctx.enter_context(tc.tile_pool(name="const", bufs=1))
    spd_p = ctx.enter_context(tc.tile_pool(name="spd", bufs=1))
    qkT_p = ctx.enter_context(tc.tile_pool(name="qkT", bufs=1))
    v_p = ctx.enter_context(tc.tile_pool(name="v", bufs=1))
    wout_p = ctx.enter_context(tc.tile_pool(name="wout", bufs=1))
    merged_p = ctx.enter_context(tc.tile_pool(name="merged", bufs=1))
    u_p = ctx.enter_context(tc.tile_pool(name="u", bufs=1))

    ones1 = const_p.tile([1, P], F32R)
    nc.sync.dma_start(ones1[:], onesr[:])
    bout_sb = const_p.tile([1, DIM], F32R)
    nc.sync.dma_start(bout_sb[:], boutr[:])

    spd_sb = spd_p.tile([P, NT, N], F32)
    nc.sync.dma_start(spd_sb[:], spd.rearrange("(t p) m -> p t m", p=P))

    # qkT_sb[p, ft, n] = (x @ Wqk)^T : f = ft*128+p ; q is ft 0..3, k is ft 4..7
    qkT_sb = qkT_p.tile([P, 2 * INNER // P, N], F32R)
    # v_sb[p, mt, :] = v[m, :] with m = mt*128+p (bf16 for the EV matmul)
    v_sb = v_p.tile([P, NT, INNER], BF16)
    wout_sb = wout_p.tile([P, KD, DIM], F32R)
    nc.sync.dma_start(wout_sb[:], woutr.rearrange("(kt p) d -> p kt d", p=P))
    # mergedT[p, s, n]: inner = s*128 + p  (head pair s = (2s, 2s+1))
    mergedT = merged_p.tile([P, KD, N], F32R)
    # u_all[:, h*8+i] = ||dots row||^2 for head h, n-tile i
    u_all = u_p.tile([P, H * NT], F32)

    # ------- stage A/B/C: projections, Gram, u -------
    with ExitStack() as early:
        xt_p = early.enter_context(tc.tile_pool(name="xt", bufs=1))
        wqk_p = early.enter_context(tc.tile_pool(name="wqk", bufs=1))
        wv_p = early.enter_context(tc.tile_pool(name="wv", bufs=1))
        kq_p = early.enter_context(tc.tile_pool(name="kqnat", bufs=1))
        g_p = early.enter_context(tc.tile_pool(name="gram", bufs=1))
        ps_proj = early.enter_context(
            tc.tile_pool(name="ps_proj", bufs=3, space="PSUM")
        )
        ps_small = early.enter_context(
            tc.tile_pool(name="ps_small", bufs=2, space="PSUM")
        )

        xt_sb = xt_p.tile([P, KD, N], F32, tag="xt")
        nc.sync.dma_start(xt_sb[:], xt.rearrange("(kt p) n -> p kt n", p=P))
        xtr_sb = xt_p.tile([P, KD, N], F32R, tag="xtr")
        nc.sync.dma_start(xtr_sb[:], xtr.rearrange("(kt p) n -> p kt n", p=P))
        wqk_sb = wqk_p.tile([P, KD, 2 * INNER], F32, tag="wqk")
        nc.sync.dma_start(wqk_sb[:], wqk.rearrange("(kt p) f -> p kt f", p=P))
        wqkr_sb = wqk_p.tile([P, KD, 2 * INNER], F32R, tag="wqkr")
        nc.sync.dma_start(wqkr_sb[:], wqkr.rearrange("(kt p) f -> p kt f", p=P))
        wv_sb = wv_p.tile([P, KD, INNER], F32R)
        nc.sync.dma_start(wv_sb[:], wvr.rearrange("(kt p) f -> p kt f", p=P))

        k_nat = kq_p.tile([P, NT, INNER], BF16, tag="knat")
        q_nat = kq_p.tile([P, NT, INNER], BF16, tag="qnat")

        # qkT (fp32 matmuls, f32r store): out[f-tile, n-chunk]
        for ft in range(2 * INNER // P):
            for ch in range(2):
                ps = ps_proj.tile([P, 512], F32)
                for kt in range(KD):
                    nc.tensor.matmul(
                        ps[:],
                        wqk_sb[:, kt, ft * P : (ft + 1) * P],
                        xt_sb[:, kt, ch * 512 : (ch + 1) * 512],
                        start=(kt == 0),
                        stop=(kt == KD - 1),
                    )
                nc.scalar.copy(qkT_sb[:, ft, ch * 512 : (ch + 1) * 512], ps[:])

        # v (bf16), k_nat, q_nat (bf16): f32r matmuls, out[m-tile, f]
        for mt in range(NT):
            ps = ps_proj.tile([P, 512], F32)
            for kt in range(KD):
                nc.tensor.matmul(
                    ps[:],
                    xtr_sb[:, kt, mt * P : (mt + 1) * P],
                    wv_sb[:, kt, :],
                    start=(kt == 0),
                    stop=(kt == KD - 1),
                )
            nc.scalar.copy(v_sb[:, mt, :], ps[:])
        for mt in range(NT):
            ps = ps_proj.tile([P, 512], F32)
            for kt in range(KD):
                nc.tensor.matmul(
                    ps[:],
                    xtr_sb[:, kt, mt * P : (mt + 1) * P],
                    wqkr_sb[:, kt, INNER : 2 * INNER],
                    start=(kt == 0),
                    stop=(kt == KD - 1),
                )
            nc.scalar.copy(k_nat[:, mt, :], ps[:])
        for mt in range(NT):
            ps = ps_proj.tile([P, 512], F32)
            for kt in range(KD):
                nc.tensor.matmul(
                    ps[:],
                    xtr_sb[:, kt, mt * P : (mt + 1) * P],
                    wqkr_sb[:, kt, 0:INNER],
                    start=(kt == 0),
                    stop=(kt == KD - 1),
                )
            nc.scalar.copy(q_nat[:, mt, :], ps[:])

        # Gram G_h = k_h^T k_h  [64, 64] (bf16 matmuls), stored f32r at
        # partitions (h%2)*64
        g_sb = g_p.tile([P, H // 2, DH], F32R)
        for h in range(H):
            base = (h % 2) * DH
            ps = ps_small.tile([P, 512], F32)
            for mt in range(NT):
                nc.tensor.matmul(
                    ps[base : base + DH, :DH],
                    k_nat[:, mt, h * DH : (h + 1) * DH],
                    k_nat[:, mt, h * DH : (h + 1) * DH],
                    start=(mt == 0),
                    stop=(mt == NT - 1),
                    tile_position=(0, base),
                )
            nc.scalar.copy(g_sb[base : base + DH, h // 2, :], ps[base : base + DH, :DH])

        # u[n] = q_n^T G q_n  via t = G^T q^T (f32r) then row-dot with q_nat
        uj_p = early.enter_context(tc.tile_pool(name="ujunk", bufs=2))
        for h in range(H):
            base = (h % 2) * DH
            for i in range(NT):
                ps = ps_small.tile([P, 512], F32)
                nc.tensor.matmul(
                    ps[:, :DH],
                    qkT_sb[base : base + DH, h // 2, i * P : (i + 1) * P],
                    g_sb[base : base + DH, h // 2, :],
                    start=True,
                    stop=True,
                )
                uj = uj_p.tile([P, DH], F32)
                nc.vector.scalar_tensor_tensor(
                    uj[:],
                    ps[:, :DH],
                    1.0,
                    q_nat[:, i, h * DH : (h + 1) * DH],
                    ALU.mult,
                    ALU.mult,
                    accum_out=u_all[:, h * NT + i : h * NT + i + 1],
                )

    dbgp = ctx.enter_context(tc.tile_pool(name="dbg", bufs=1))
    nc.sync.dma_start(DBG["o_qkT"][:], qkT_sb[:].bitcast(F32))
    nc.sync.dma_start(DBG["o_v"][:], v_sb[:])
    nc.sync.dma_start(DBG["o_u"][:], u_all[:])
    nc.sync.dma_start(DBG["o_g"][:], g_sb[:].bitcast(F32))

    # ------- stage D: scores per head -------
    score_ctx = ExitStack()
    ctx.enter_context(score_ctx)
    Ph_p = score_ctx.enter_context(tc.tile_pool(name="Phead", bufs=1))
    z_p = score_ctx.enter_context(tc.tile_pool(name="z", bufs=2))
    e_p = score_ctx.enter_context(tc.tile_pool(name="E", bufs=3))
    et_p = score_ctx.enter_context(tc.tile_pool(name="ET", bufs=2))
    sq_p = score_ctx.enter_context(tc.tile_pool(name="sqjunk", bufs=2))
    st_p = score_ctx.enter_context(tc.tile_pool(name="stats", bufs=2))
    dr_p = score_ctx.enter_context(tc.tile_pool(name="denr", bufs=8))
    ps_dots = score_ctx.enter_context(
        tc.tile_pool(name="ps_dots", bufs=4, space="PSUM")
    )
    ps_ev = score_ctx.enter_context(tc.tile_pool(name="ps_ev", bufs=2, space="PSUM"))

    def qk_mm(ps, h, i, ch):
        base = (h % 2) * DH
        nc.tensor.matmul(
            ps[:],
            qkT_sb[base : base + DH, h // 2, i * P : (i + 1) * P],
            qkT_sb[base : base + DH, 4 + h // 2, ch * 512 : (ch + 1) * 512],
            start=True,
            stop=True,
        )

    et_tiles = {}
    for h in range(H):
        # pass 1: P = dots*spd, row sums of P^2 via ACT square-accumulate
        Ph = Ph_p.tile([P, NT, N], F32)
        ssq = st_p.tile([P, NT], F32, tag="ssq")
        for i in range(NT):
            for ch in range(2):
                ps = ps_dots.tile([P, 512], F32)
                qk_mm(ps, h, i, ch)
                nc.vector.scalar_tensor_tensor(
                    Ph[:, i, ch * 512 : (ch + 1) * 512],
                    ps[:],
                    1.0,
                    spd_sb[:, i, ch * 512 : (ch + 1) * 512],
                    ALU.mult,
                    ALU.mult,
                )
            sqj = sq_p.tile([P, N], F32)
            nc.scalar.activation(
                sqj[:],
                Ph[:, i, :],
                AF.Square,
                accum_out=ssq[:, i : i + 1],
            )
        # rho = exp(0.5*(ln u - ln max(ssq, 1e-24)))   [P, NT]
        s1 = st_p.tile([P, NT], F32, tag="s1")
        nc.vector.tensor_scalar_max(s1[:], ssq[:], 1e-24)
        s2 = st_p.tile([P, NT], F32, tag="s2")
        nc.scalar.activation(s2[:], s1[:], AF.Ln)
        s3 = st_p.tile([P, NT], F32, tag="s3")
        nc.scalar.activation(s3[:], u_all[:, h * NT : (h + 1) * NT], AF.Ln)
        s4 = st_p.tile([P, NT], F32, tag="s4")
        nc.vector.tensor_tensor(s4[:], s3[:], s2[:], ALU.subtract)
        rho = st_p.tile([P, NT], F32, tag="rho")
        nc.scalar.activation(rho[:], s4[:], AF.Exp, scale=0.5)


        # pass 2: z = P*rho + dots (dots recomputed), E = exp(z), E /= den
        et = et_p.tile([P, NT, N], BF16)
        et_tiles[h] = et
        for i in range(NT):
            zt = z_p.tile([P, N], F32)
            for ch in range(2):
                ps = ps_dots.tile([P, 512], F32)
                qk_mm(ps, h, i, ch)
                nc.vector.scalar_tensor_tensor(
                    zt[:, ch * 512 : (ch + 1) * 512],
                    Ph[:, i, ch * 512 : (ch + 1) * 512],
                    rho[:, i : i + 1],
                    ps[:],
                    ALU.mult,
                    ALU.add,
                )
            den = dr_p.tile([P, 1], F32, tag="den")
            et_src = e_p.tile([P, N], BF16)
            nc.scalar.activation(et_src[:], zt[:], AF.Exp, accum_out=den[:])
            rden = dr_p.tile([P, 1], F32, tag="rden")
            nc.vector.reciprocal(rden[:], den[:])
            nc.vector.tensor_scalar_mul(et_src[:], et_src[:], rden[:])


            nc.sync.dma_start_transpose(et[:, :, i * P : (i + 1) * P], et_src[:])

        # EV for the completed pair (bf16)
        if h % 2 == 1:
            for ch in range(2):
                ps = ps_ev.tile([P, 512], F32)
                for hh in (h - 1, h):
                    base = (hh % 2) * DH
                    for mt in range(NT):
                        nc.tensor.matmul(
                            ps[base : base + DH, :],
                            v_sb[:, mt, hh * DH : (hh + 1) * DH],
                            et_tiles[hh][:, mt, ch * 512 : (ch + 1) * 512],
                            start=(mt == 0),
                            stop=(mt == NT - 1),
                            tile_position=(0, base),
                        )
                nc.scalar.copy(
                    mergedT[:, h // 2, ch * 512 : (ch + 1) * 512], ps[:]
                )

            et_tiles.clear()

    score_ctx.close()


    # ------- stage E: output projection + bias (f32r) -------
    with ExitStack() as fin:
        ps_out = fin.enter_context(tc.tile_pool(name="ps_out", bufs=2, space="PSUM"))
        yo_p = fin.enter_context(tc.tile_pool(name="yout", bufs=2))
        for i in range(NT):
            ps = ps_out.tile([P, DIM], F32)
            nc.tensor.matmul(ps[:], ones1[:, :], bout_sb[:, :], start=True, stop=False)
            for kt in range(KD):
                nc.tensor.matmul(
                    ps[:],
                    mergedT[:, kt, i * P : (i + 1) * P],
                    wout_sb[:, kt, :],
                    start=False,
                    stop=(kt == KD - 1),
                )
            yo = yo_p.tile([P, DIM], F32)
            nc.scalar.copy(yo[:], ps[:])
            nc.sync.dma_start(y[i * P : (i + 1) * P, :], yo[:])


_NC_CACHE = None


def _get_nc():
    global _NC_CACHE
    if _NC_CACHE is None:
        _NC_CACHE = _build_bass()
    return _NC_CACHE


def _in_maps(x, spd, head_mask, W_qkv, W_out, b_out):
    wqk = W_qkv[:, : 2 * INNER].copy()
    wqk[:, INNER:] *= SCALE                     # dots scale into k
    wv = np.ascontiguousarray(W_qkv[:, 2 * INNER :])
    scale_m = head_mask * (H / head_mask.sum())  # head-dropout rescale
    wout = np.ascontiguousarray(W_out * np.repeat(scale_m, DH)[:, None])
    bout = np.ascontiguousarray(b_out.reshape(1, DIM))

    maps = []
    for b in range(B):
        xb = np.ascontiguousarray(x[b].T)
        maps.append(
            {
                "xt": xb,
                "xtr": xb,
                "spd": np.ascontiguousarray(spd[b, 0]),
                "wqk": wqk,
                "wqkr": wqk,
                "wvr": wv,
                "woutr": wout,
                "boutr": bout,
                "onesr": np.ones((1, P), dtype=np.float32),
            }
        )
    return maps


def kernel(x, spd, head_mask, W_qkv, W_out, b_out):
    x = np.asarray(x, dtype=np.float32)
    spd = np.asarray(spd, dtype=np.float32)
    head_mask = np.asarray(head_mask, dtype=np.float32)
    W_qkv = np.asarray(W_qkv, dtype=np.float32)
    W_out = np.asarray(W_out, dtype=np.float32)
    b_out = np.asarray(b_out, dtype=np.float32)

    nc = _get_nc()
    res = run_bass_kernel_spmd(
        nc, _in_maps(x, spd, head_mask, W_qkv, W_out, b_out),
        core_ids=list(range(B)),
    )
    return np.stack([res.results[b]["y"] for b in range(B)], axis=0)
